# revision 2
# baseline (speedup 1.0000x reference)
import numpy as np
import concourse.bass as bass
import concourse.tile as tile
from concourse import bacc, mybir
from concourse.bass_utils import run_bass_kernel_spmd

# Block self-attention: 32x32 areas of 4x4 blocks of 8x8 pixels.
# Sharding: 8 cores = 4 batches x 2 H-halves of 256 rows (8 area-rows).
# Padding trick: host pads x spatially with the vector xpad solving
# w_ptg @ xpad + b_ptg = 0, so conv1 output is exactly 0 at padded pixels
# (matches reference, which zero-pads after conv+bias).

F32 = mybir.dt.float32
F32R = mybir.dt.float32r
MM_DT = F32

_cached = {}


def _build_nc():
    nc = bacc.Bacc("TRN2", target_bir_lowering=False, debug=False, num_devices=8)
    xs = nc.dram_tensor("xs", [64, 256, 512], F32, kind="ExternalInput").ap()
    w1t = nc.dram_tensor("w1t", [64, 48], F32, kind="ExternalInput").ap()
    b1 = nc.dram_tensor("b1", [48, 1], F32, kind="ExternalInput").ap()
    w2t = nc.dram_tensor("w2t", [16, 64], F32, kind="ExternalInput").ap()
    b2 = nc.dram_tensor("b2", [64, 1], F32, kind="ExternalInput").ap()
    ident = nc.dram_tensor("ident", [128, 128], F32, kind="ExternalInput").ap()
    mask = nc.dram_tensor("mask", [128, 128], F32, kind="ExternalInput").ap()
    out = nc.dram_tensor("out", [64, 256, 512], F32, kind="ExternalOutput").ap()

    def mm(o, l, r, **kw):
        if MM_DT is not F32:
            l = l.bitcast(MM_DT)
            r = r.bitcast(MM_DT)
        nc.tensor.matmul(o, l, r, **kw)

    with tile.TileContext(nc) as tc:
        with (
            tc.tile_pool(name="const", bufs=1) as cpool,
            tc.tile_pool(name="xy", bufs=1) as xy,
            tc.tile_pool(name="mid", bufs=2) as mid,
            tc.tile_pool(name="sm", bufs=3) as sm,
            tc.tile_pool(name="ps", bufs=3, space="PSUM") as ps,
        ):
            w1_t = cpool.tile([64, 48], F32)
            nc.sync.dma_start(w1_t, w1t)
            b1_t = cpool.tile([48, 1], F32)
            nc.sync.dma_start(b1_t, b1)
            w2_t = cpool.tile([16, 64], F32)
            nc.sync.dma_start(w2_t, w2t)
            b2_t = cpool.tile([64, 1], F32)
            nc.sync.dma_start(b2_t, b2)
            id_t = cpool.tile([128, 128], F32)
            nc.sync.dma_start(id_t, ident)
            mk_t = cpool.tile([128, 128], F32)
            nc.sync.dma_start(mk_t, mask)

            for s in range(8):
                for gw in range(2):
                    # half strip: 32 rows x 256 cols = 8 areas
                    x_t = xy.tile([64, 32 * 256], F32, tag="x")
                    nc.sync.dma_start(
                        x_t,
                        xs[:, 32 * s : 32 * s + 32, 256 * gw : 256 * gw + 256],
                    )
                    # block-ordered view: (c, ih, aw, iw, ph, pw)
                    xv = x_t.rearrange(
                        "c (ih ph aw iw pw) -> c ih aw iw ph pw",
                        ih=4, ph=8, aw=8, iw=4, pw=8,
                    )
                    y_t = xy.tile([48, 8192], F32, tag="y")
                    for a in range(8):
                        for ih in range(4):
                            p1 = ps.tile([48, 256], F32, tag="mm")
                            mm(p1, w1_t, xv[:, ih, a], start=True, stop=True)
                            off = a * 1024 + ih * 256
                            nc.scalar.activation(
                                y_t[:, off : off + 256],
                                p1, mybir.ActivationFunctionType.Identity, bias=b1_t,
                            )
                    # y free index = aw*1024 + i*64 + p (block-linear)
                    qkc = mid.tile([128, 2048], F32, tag="qkc")
                    qv = qkc.rearrange("ai (c p) -> ai c p", c=32, p=64)
                    for c in range(32):
                        nc.sync.dma_start(qv[:, c], y_t[c : c + 1, :])
                    gc = mid.tile([128, 1024], F32, tag="gc")
                    gv = gc.rearrange("ai (c p) -> ai c p", c=16, p=64)
                    for c in range(16):
                        nc.sync.dma_start(gv[:, c], y_t[32 + c : 33 + c, :])
                    qkb = mid.tile([128, 2048], F32, tag="qkb")
                    for q in range(16):
                        tp = ps.tile([128, 128], F32, tag="tp")
                        nc.tensor.transpose(tp, qkc[:, 128 * q : 128 * q + 128], id_t)
                        sl = qkb[:, 128 * q : 128 * q + 128]
                        if q % 2 == 0:
                            nc.scalar.activation(
                                sl, tp, mybir.ActivationFunctionType.Copy
                            )
                        else:
                            nc.vector.tensor_copy(sl, tp)
                    sps = ps.tile([128, 128], F32, tag="mm")
                    for k in range(8):
                        mm(
                            sps,
                            qkb[:, 128 * k : 128 * k + 128],
                            qkb[:, 1024 + 128 * k : 1024 + 128 * k + 128],
                            start=(k == 0), stop=(k == 7),
                        )
                    e_t = sm.tile([128, 128], F32, tag="e")
                    nc.scalar.activation(e_t, sps, mybir.ActivationFunctionType.Exp)
                    nc.vector.tensor_mul(e_t, e_t, mk_t)
                    r_t = sm.tile([128, 1], F32, tag="r")
                    nc.vector.reduce_sum(r_t, e_t, axis=mybir.AxisListType.X)
                    nc.vector.reciprocal(r_t, r_t)
                    p_t = sm.tile([128, 128], F32, tag="p")
                    nc.vector.tensor_scalar_mul(p_t, e_t, r_t)
                    ptp = ps.tile([128, 128], F32, tag="tp")
                    nc.tensor.transpose(ptp, p_t, id_t)
                    pT = sm.tile([128, 128], F32, tag="pT")
                    nc.scalar.activation(pT, ptp, mybir.ActivationFunctionType.Copy)
                    o_c = mid.tile([128, 1024], F32, tag="oc")
                    for h in range(2):
                        op = ps.tile([128, 512], F32, tag="mm")
                        mm(op, pT, gc[:, 512 * h : 512 * h + 512],
                           start=True, stop=True)
                        sl = o_c[:, 512 * h : 512 * h + 512]
                        if h == 0:
                            nc.scalar.activation(
                                sl, op, mybir.ActivationFunctionType.Copy
                            )
                        else:
                            nc.vector.tensor_copy(sl, op)
                    ost = xy.tile([16, 8192], F32, tag="ost")
                    ocv = o_c.rearrange("ai (c p) -> ai c p", c=16, p=64)
                    for c in range(16):
                        nc.sync.dma_start(ost[c : c + 1], ocv[:, c])
                    osum = xy.tile([64, 8192], F32, tag="osum")
                    # image-ordered view: flat = (ih*8+ph)*256 + a*32 + iw*8 + pw
                    osv = osum.rearrange(
                        "c (ih ph aw iw pw) -> c ih aw ph iw pw",
                        ih=4, ph=8, aw=8, iw=4, pw=8,
                    )
                    for a in range(8):
                        for ih in range(4):
                            off = a * 1024 + ih * 256
                            p2 = ps.tile([64, 256], F32, tag="mm")
                            mm(p2, w2_t, ost[:, off : off + 256],
                               start=True, stop=False)
                            mm(p2, id_t[0:64, 0:64], xv[:, ih, a],
                               start=False, stop=True)
                            p2v = p2.rearrange(
                                "c (iw ph pw) -> c ph iw pw", iw=4, ph=8, pw=8
                            )
                            nc.scalar.activation(
                                osv[:, ih, a],
                                p2v, mybir.ActivationFunctionType.Identity, bias=b2_t,
                            )
                    nc.sync.dma_start(
                        out[:, 32 * s : 32 * s + 32, 256 * gw : 256 * gw + 256],
                        osum,
                    )
    nc.compile()
    return nc


def kernel(x, w_ptg, b_ptg, w_out, b_out):
    x = np.asarray(x, dtype=np.float32)
    w_ptg = np.asarray(w_ptg, dtype=np.float32)
    b_ptg = np.asarray(b_ptg, dtype=np.float32)
    w_out = np.asarray(w_out, dtype=np.float32)
    b_out = np.asarray(b_out, dtype=np.float32)

    # pad vector: w_ptg @ xpad + b_ptg = 0
    xpad, *_ = np.linalg.lstsq(w_ptg, -b_ptg, rcond=None)
    xp = np.empty((4, 64, 512, 512), np.float32)
    xp[:] = xpad.astype(np.float32)[None, :, None, None]
    xp[:, :, :504, :504] = x

    ident = np.eye(128, dtype=np.float32)
    mask = np.zeros((128, 128), np.float32)
    for a in range(8):
        mask[16 * a : 16 * a + 16, 16 * a : 16 * a + 16] = 1.0

    common = {
        "w1t": np.ascontiguousarray(w_ptg.T),
        "b1": np.ascontiguousarray(b_ptg[:, None]),
        "w2t": np.ascontiguousarray(w_out.T),
        "b2": np.ascontiguousarray(b_out[:, None]),
        "ident": ident,
        "mask": mask,
    }
    in_maps = []
    for b in range(4):
        for h in range(2):
            in_maps.append(
                {"xs": np.ascontiguousarray(xp[b, :, 256 * h : 256 * h + 256, :]),
                 **common}
            )

    if "nc" not in _cached:
        _cached["nc"] = _build_nc()
    res = run_bass_kernel_spmd(_cached["nc"], in_maps, list(range(8)))
    _cached["last_res"] = res

    outp = np.empty((4, 64, 512, 512), np.float32)
    for i in range(8):
        b, h = divmod(i, 2)
        outp[b, :, 256 * h : 256 * h + 256, :] = res.results[i]["out"]
    return np.ascontiguousarray(outp[:, :, :504, :504])


if __name__ == "__main__":
    import reference

    inputs = {k: np.asarray(v) for k, v in reference.setup_inputs().items()}
    got = kernel(**inputs)
    exp = np.asarray(reference.reference(**inputs))
    err = np.abs(got - exp).max() / np.abs(exp).max()
    print("Relative error:", err)



# revision 17
# speedup vs baseline: 3.3682x; 3.3682x over previous
import numpy as np
import ml_dtypes
import concourse.bass as bass
import concourse.tile as tile
from concourse import bacc, mybir
from concourse.bass_utils import run_bass_kernel_spmd

# Block self-attention: 32x32 areas of 4x4 blocks of 8x8 pixels.
# Sharding: 8 cores = 4 batches x 2 H-halves of 256 rows.
# Host passes x in block-major bf16 layout [64, 16 units x 8192] where a
# unit is 8 areas (one 32-row x 256-col half strip) laid out (area, block,
# pixel). Kernel returns conv2(attention(conv1(x))) in the same layout;
# host adds the residual x and the output bias b2, then un-permutes.
# Padding trick: host pads x spatially with the vector xpad solving
# w_ptg @ xpad + b_ptg = 0, so conv1 output is exactly 0 at padded pixels
# (matches reference, which zero-pads after conv+bias).
# Softmax mask (block-diagonal per area) is applied by accumulating
# -1e4 + 1e4*blockdiag onto the scores via two extra rank-1/rank-8
# matmuls before the exp.

F32 = mybir.dt.float32
BF16 = mybir.dt.bfloat16
AF = mybir.ActivationFunctionType
BF = ml_dtypes.bfloat16

_cached = {}


def _build_nc(units=16):
    nc = bacc.Bacc("TRN2", target_bir_lowering=False, debug=False, num_devices=8)
    xs = nc.dram_tensor("xs", [64, units * 8192], BF16, kind="ExternalInput").ap()
    w1t = nc.dram_tensor("w1t", [64, 48], BF16, kind="ExternalInput").ap()
    b1 = nc.dram_tensor("b1", [48, 1], F32, kind="ExternalInput").ap()
    w2t = nc.dram_tensor("w2t", [16, 64], BF16, kind="ExternalInput").ap()
    identb = nc.dram_tensor("identb", [128, 128], BF16, kind="ExternalInput").ap()
    mbu = nc.dram_tensor("mbu", [1, 128], BF16, kind="ExternalInput").ap()
    mbv = nc.dram_tensor("mbv", [1, 128], BF16, kind="ExternalInput").ap()
    mbc = nc.dram_tensor("mbc", [8, 128], BF16, kind="ExternalInput").ap()
    out = nc.dram_tensor("out", [64, units * 8192], BF16, kind="ExternalOutput").ap()

    with tile.TileContext(nc) as tc:
        with (
            tc.tile_pool(name="const", bufs=1) as cpool,
            tc.tile_pool(name="xio", bufs=2) as xio,
            tc.tile_pool(name="stage", bufs=2) as stg,
            tc.tile_pool(name="sm", bufs=2) as sm,
            tc.tile_pool(name="dbounce", bufs=2, space="DRAM") as dpool,
            tc.tile_pool(name="pcv", bufs=2, space="PSUM") as pcv,
            tc.tile_pool(name="ptp", bufs=2, space="PSUM") as ptp,
            tc.tile_pool(name="patt", bufs=2, space="PSUM") as patt,
        ):
            w1_t = cpool.tile([64, 48], BF16)
            nc.sync.dma_start(w1_t, w1t)
            b1_t = cpool.tile([48, 1], F32)
            nc.sync.dma_start(b1_t, b1)
            w2_t = cpool.tile([16, 64], BF16)
            nc.sync.dma_start(w2_t, w2t)
            id_t = cpool.tile([128, 128], BF16)
            nc.sync.dma_start(id_t, identb)
            mbu_t = cpool.tile([1, 128], BF16)
            nc.sync.dma_start(mbu_t, mbu)
            mbv_t = cpool.tile([1, 128], BF16)
            nc.sync.dma_start(mbv_t, mbv)
            mbc_t = cpool.tile([8, 128], BF16)
            nc.sync.dma_start(mbc_t, mbc)

            for u in range(units):
                x_t = xio.tile([64, 8192], BF16, tag="x")
                nc.sync.dma_start(x_t, xs[:, 8192 * u : 8192 * (u + 1)])

                # conv1: y = w1 @ x (+b1 in the copy), bf16, block-pixel order
                y_t = stg.tile([48, 8192], BF16, tag="y")
                for t in range(8):
                    c1 = pcv.tile([64, 1024], F32, tag="cv")
                    for hh in range(2):
                        nc.tensor.matmul(
                            c1[0:48, 512 * hh : 512 * hh + 512],
                            w1_t,
                            x_t[:, 1024 * t + 512 * hh : 1024 * t + 512 * hh + 512],
                            start=True,
                            stop=True,
                        )
                    ysl = y_t[:, 1024 * t : 1024 * t + 1024]
                    if t % 2 == 0:
                        nc.scalar.activation(
                            ysl, c1[0:48, :], AF.Identity, bias=b1_t
                        )
                    else:
                        nc.vector.tensor_scalar_add(ysl, c1[0:48, :], b1_t)

                # gather y [c,(a,i,p)] -> qg [(a,i),(c,p)] ; c: 0-31 q/t, 32-47 g
                # Two hops via a DRAM bounce buffer: SBUF-side APs must keep
                # the partition dim outermost (partition-interior APs lower
                # incorrectly), while DRAM APs are flat and take any stride
                # pattern. Hop 1 is a contiguous spill; hop 2 permutes on the
                # DRAM read side (long writes, 128B reads - no HBM RMW).
                yb = dpool.tile([48, 8192], BF16, tag="yb")
                nc.scalar.dma_start(yb, y_t)
                qg = stg.tile([128, 3072], BF16, tag="qg")
                nc.scalar.dma_start(
                    qg,
                    yb.rearrange("c (a i p) -> (a i) c p", a=8, i=16, p=64),
                )

                # transpose q/t chunks: qkb [(c,p), ai]
                qkb = stg.tile([128, 2048], BF16, tag="qkb")
                for g4 in range(4):
                    tp = ptp.tile([128, 512], BF16, tag="tp")
                    for q in range(4):
                        nc.tensor.transpose(
                            tp[:, 128 * q : 128 * q + 128],
                            qg[:, 512 * g4 + 128 * q : 512 * g4 + 128 * q + 128],
                            id_t,
                        )
                    dst = qkb[:, 512 * g4 : 512 * g4 + 512]
                    if g4 % 2 == 0:
                        nc.vector.tensor_copy(dst, tp)
                    else:
                        nc.scalar.activation(dst, tp, AF.Copy)

                # scores (8 areas batched; mask applied via bias matmuls)
                sps = patt.tile([128, 512], F32, tag="att")
                s_ap = sps[:, 0:128]
                for k in range(8):
                    nc.tensor.matmul(
                        s_ap,
                        qkb[:, 128 * k : 128 * k + 128],
                        qkb[:, 1024 + 128 * k : 1024 + 128 * k + 128],
                        start=(k == 0),
                        stop=False,
                    )
                nc.tensor.matmul(s_ap, mbu_t, mbv_t, start=False, stop=False)
                nc.tensor.matmul(s_ap, mbc_t, mbc_t, start=False, stop=True)

                e_t = sm.tile([128, 128], BF16, tag="e")
                nc.scalar.activation(e_t, s_ap, AF.Exp)
                r_t = sm.tile([128, 1], F32, tag="r")
                nc.vector.reduce_sum(r_t, e_t, axis=mybir.AxisListType.X)
                nc.vector.reciprocal(r_t, r_t)

                etp = ptp.tile([128, 512], BF16, tag="tp")
                nc.tensor.transpose(etp[:, 0:128], e_t, id_t)
                eT = sm.tile([128, 128], BF16, tag="eT")
                nc.vector.tensor_copy(eT, etp[:, 0:128])

                # o = (e @ g) * recip(rowsum) ; normalization in the copy
                o_c = stg.tile([128, 1024], BF16, tag="oc")
                for hh in range(2):
                    op = patt.tile([128, 512], F32, tag="att")
                    nc.tensor.matmul(
                        op,
                        eT,
                        qg[:, 2048 + 512 * hh : 2048 + 512 * hh + 512],
                        start=True,
                        stop=True,
                    )
                    osl = o_c[:, 512 * hh : 512 * hh + 512]
                    if hh == 0:
                        nc.vector.tensor_scalar_mul(osl, op, r_t)
                    else:
                        nc.scalar.activation(osl, op, AF.Copy, scale=r_t)

                # scatter o_c [(a,i),(c,p)] -> ost [c,(a,i,p)] via DRAM bounce
                # (contiguous spill, permuted read)
                ob = dpool.tile([128, 1024], BF16, tag="ob")
                nc.scalar.dma_start(ob, o_c)
                ost = stg.tile([16, 8192], BF16, tag="ost")
                nc.scalar.dma_start(
                    ost,
                    ob.rearrange("(a i) (c p) -> c (a i) p", a=8, i=16, c=16, p=64),
                )

                # conv2 (residual + b2 added on host)
                osum = xio.tile([64, 8192], BF16, tag="os")
                for t in range(8):
                    c2 = pcv.tile([64, 1024], F32, tag="cv")
                    for hh in range(2):
                        nc.tensor.matmul(
                            c2[:, 512 * hh : 512 * hh + 512],
                            w2_t,
                            ost[:, 1024 * t + 512 * hh : 1024 * t + 512 * hh + 512],
                            start=True,
                            stop=True,
                        )
                    osl = osum[:, 1024 * t : 1024 * t + 1024]
                    if t % 2 == 0:
                        nc.vector.tensor_copy(osl, c2)
                    else:
                        nc.scalar.activation(osl, c2, AF.Copy)

                nc.sync.dma_start(out[:, 8192 * u : 8192 * (u + 1)], osum)
    nc.compile()
    return nc


def kernel(x, w_ptg, b_ptg, w_out, b_out):
    x = np.asarray(x, dtype=np.float32)
    w_ptg = np.asarray(w_ptg, dtype=np.float32)
    b_ptg = np.asarray(b_ptg, dtype=np.float32)
    w_out = np.asarray(w_out, dtype=np.float32)
    b_out = np.asarray(b_out, dtype=np.float32)

    # pad vector: w_ptg @ xpad + b_ptg = 0
    xpad, *_ = np.linalg.lstsq(w_ptg, -b_ptg, rcond=None)
    xp = np.empty((4, 64, 512, 512), np.float32)
    xp[:] = xpad.astype(np.float32)[None, :, None, None]
    xp[:, :, :504, :504] = x

    mc = np.zeros((8, 128), dtype=BF)
    for a in range(8):
        mc[a, 16 * a : 16 * a + 16] = 100.0

    common = {
        "w1t": np.ascontiguousarray(w_ptg.T).astype(BF),
        "b1": np.ascontiguousarray(b_ptg[:, None]).astype(np.float32),
        "w2t": np.ascontiguousarray(w_out.T).astype(BF),
        "identb": np.eye(128, dtype=BF),
        "mbu": np.full((1, 128), -100.0, dtype=BF),
        "mbv": np.full((1, 128), 100.0, dtype=BF),
        "mbc": mc,
    }
    in_maps = []
    for b in range(4):
        for h in range(2):
            xb = xp[b, :, 256 * h : 256 * h + 256, :].reshape(
                64, 8, 4, 8, 2, 8, 4, 8
            )
            xb = xb.transpose(0, 1, 4, 5, 2, 6, 3, 7).reshape(64, 131072)
            in_maps.append({"xs": np.ascontiguousarray(xb.astype(BF)), **common})

    if "nc" not in _cached:
        _cached["nc"] = _build_nc()
    res = run_bass_kernel_spmd(_cached["nc"], in_maps, list(range(8)))
    _cached["last_res"] = res

    outp = np.empty((4, 64, 512, 512), np.float32)
    for idx in range(8):
        b, h = divmod(idx, 2)
        o = np.asarray(res.results[idx]["out"]).astype(np.float32)
        o = (
            o.reshape(64, 8, 2, 8, 4, 4, 8, 8)
            .transpose(0, 1, 4, 6, 2, 3, 5, 7)
            .reshape(64, 256, 512)
        )
        outp[b, :, 256 * h : 256 * h + 256, :] = o
    out = outp[:, :, :504, :504] + b_out[None, :, None, None] + x
    return np.ascontiguousarray(out.astype(np.float32))


if __name__ == "__main__":
    import reference

    inputs = {k: np.asarray(v) for k, v in reference.setup_inputs().items()}
    got = kernel(**inputs)
    exp = np.asarray(reference.reference(**inputs))
    err = np.abs(got - exp).max() / np.abs(exp).max()
    print("Relative error:", err)


# revision 18
# speedup vs baseline: 5.8315x; 1.7313x over previous
import numpy as np
import ml_dtypes
import concourse.bass as bass
import concourse.tile as tile
from concourse import bacc, mybir
from concourse.bass_utils import run_bass_kernel_spmd

# Block self-attention: 32x32 areas of 4x4 blocks of 8x8 pixels.
# Sharding: 8 cores = 4 batches x 2 H-halves of 256 rows.
# Host passes x in block-major bf16 layout [64, 16 units x 8192] where a
# unit is 8 areas (one 32-row x 256-col half strip) laid out (area, block,
# pixel). Kernel returns conv2(attention(conv1(x))) in the same layout;
# host adds the residual x and the output bias b2, then un-permutes.
# Padding trick: host pads x spatially with the vector xpad solving
# w_ptg @ xpad + b_ptg = 0, so conv1 output is exactly 0 at padded pixels
# (matches reference, which zero-pads after conv+bias).
# Softmax mask (block-diagonal per area) is applied by accumulating
# -1e4 + 1e4*blockdiag onto the scores via two extra rank-1/rank-8
# matmuls before the exp.

F32 = mybir.dt.float32
BF16 = mybir.dt.bfloat16
AF = mybir.ActivationFunctionType
BF = ml_dtypes.bfloat16

_cached = {}


def _build_nc(units=16):
    nc = bacc.Bacc("TRN2", target_bir_lowering=False, debug=False, num_devices=8)
    xs = nc.dram_tensor("xs", [64, units * 8192], BF16, kind="ExternalInput").ap()
    w1t = nc.dram_tensor("w1t", [64, 48], BF16, kind="ExternalInput").ap()
    b1 = nc.dram_tensor("b1", [48, 1], F32, kind="ExternalInput").ap()
    w2t = nc.dram_tensor("w2t", [16, 64], BF16, kind="ExternalInput").ap()
    identb = nc.dram_tensor("identb", [128, 128], BF16, kind="ExternalInput").ap()
    mbu = nc.dram_tensor("mbu", [1, 128], BF16, kind="ExternalInput").ap()
    mbv = nc.dram_tensor("mbv", [1, 128], BF16, kind="ExternalInput").ap()
    mbc = nc.dram_tensor("mbc", [8, 128], BF16, kind="ExternalInput").ap()
    out = nc.dram_tensor("out", [64, units * 8192], BF16, kind="ExternalOutput").ap()

    with tile.TileContext(nc) as tc:
        with (
            tc.tile_pool(name="const", bufs=1) as cpool,
            tc.tile_pool(name="xio", bufs=2) as xio,
            tc.tile_pool(name="stage", bufs=2) as stg,
            tc.tile_pool(name="sm", bufs=2) as sm,
            tc.tile_pool(name="dbounce", bufs=2, space="DRAM") as dpool,
            tc.tile_pool(name="pcv", bufs=2, space="PSUM") as pcv,
            tc.tile_pool(name="ptp", bufs=2, space="PSUM") as ptp,
            tc.tile_pool(name="patt", bufs=2, space="PSUM") as patt,
        ):
            w1_t = cpool.tile([64, 48], BF16)
            nc.sync.dma_start(w1_t, w1t)
            b1_t = cpool.tile([48, 1], F32)
            nc.sync.dma_start(b1_t, b1)
            w2_t = cpool.tile([16, 64], BF16)
            nc.sync.dma_start(w2_t, w2t)
            id_t = cpool.tile([128, 128], BF16)
            nc.sync.dma_start(id_t, identb)
            mbu_t = cpool.tile([1, 128], BF16)
            nc.sync.dma_start(mbu_t, mbu)
            mbv_t = cpool.tile([1, 128], BF16)
            nc.sync.dma_start(mbv_t, mbv)
            mbc_t = cpool.tile([8, 128], BF16)
            nc.sync.dma_start(mbc_t, mbc)

            state = {}

            def stage_in(u):
                x_t = xio.tile([64, 8192], BF16, tag="x")
                nc.sync.dma_start(x_t, xs[:, 8192 * u : 8192 * (u + 1)])
                state[("x", u)] = x_t

            def stage_a(u):
                # conv1: y = w1 @ x (+b1 in the copy), bf16
                x_t = state.pop(("x", u))
                y_t = stg.tile([48, 8192], BF16, tag="y")
                for t in range(8):
                    c1 = pcv.tile([64, 1024], F32, tag="cv")
                    for hh in range(2):
                        nc.tensor.matmul(
                            c1[0:48, 512 * hh : 512 * hh + 512],
                            w1_t,
                            x_t[:, 1024 * t + 512 * hh : 1024 * t + 512 * hh + 512],
                            start=True,
                            stop=True,
                        )
                    ysl = y_t[:, 1024 * t : 1024 * t + 1024]
                    if t % 2 == 0:
                        nc.scalar.activation(ysl, c1[0:48, :], AF.Identity, bias=b1_t)
                    else:
                        nc.vector.tensor_scalar_add(ysl, c1[0:48, :], b1_t)

                # gather y [c,(a,i,p)] -> qg [(a,i),(c,p)] via DRAM bounce:
                # contiguous spill (scalar ring), permuted read (sync ring).
                # SBUF-side DMA APs must keep the partition dim outermost;
                # DRAM APs are flat and take any stride pattern.
                yb = dpool.tile([48, 8192], BF16, tag="yb")
                nc.scalar.dma_start(yb, y_t)
                qg = stg.tile([128, 3072], BF16, tag="qg")
                nc.sync.dma_start(
                    qg,
                    yb.rearrange("c (a i p) -> (a i) c p", a=8, i=16, p=64),
                )
                state[("qg", u)] = qg

            def stage_b(u):
                qg = state.pop(("qg", u))
                # transpose q/t chunks: qkb [(c,p), ai]
                qkb = stg.tile([128, 2048], BF16, tag="qkb")
                for g4 in range(4):
                    tp = ptp.tile([128, 512], BF16, tag="tp")
                    for q in range(4):
                        nc.tensor.transpose(
                            tp[:, 128 * q : 128 * q + 128],
                            qg[:, 512 * g4 + 128 * q : 512 * g4 + 128 * q + 128],
                            id_t,
                        )
                    dst = qkb[:, 512 * g4 : 512 * g4 + 512]
                    if g4 % 2 == 0:
                        nc.vector.tensor_copy(dst, tp)
                    else:
                        nc.scalar.activation(dst, tp, AF.Copy)

                # scores (8 areas batched; mask applied via bias matmuls)
                sps = patt.tile([128, 512], F32, tag="att")
                s_ap = sps[:, 0:128]
                for k in range(8):
                    nc.tensor.matmul(
                        s_ap,
                        qkb[:, 128 * k : 128 * k + 128],
                        qkb[:, 1024 + 128 * k : 1024 + 128 * k + 128],
                        start=(k == 0),
                        stop=False,
                    )
                nc.tensor.matmul(s_ap, mbu_t, mbv_t, start=False, stop=False)
                nc.tensor.matmul(s_ap, mbc_t, mbc_t, start=False, stop=True)

                e_t = sm.tile([128, 128], BF16, tag="e")
                nc.scalar.activation(e_t, s_ap, AF.Exp)
                r_t = sm.tile([128, 1], F32, tag="r")
                nc.vector.reduce_sum(r_t, e_t, axis=mybir.AxisListType.X)
                nc.vector.reciprocal(r_t, r_t)

                etp = ptp.tile([128, 512], BF16, tag="tp")
                nc.tensor.transpose(etp[:, 0:128], e_t, id_t)
                eT = sm.tile([128, 128], BF16, tag="eT")
                nc.vector.tensor_copy(eT, etp[:, 0:128])

                # o = (e @ g) * recip(rowsum) ; normalization in the copy
                o_c = stg.tile([128, 1024], BF16, tag="oc")
                for hh in range(2):
                    op = patt.tile([128, 512], F32, tag="att")
                    nc.tensor.matmul(
                        op,
                        eT,
                        qg[:, 2048 + 512 * hh : 2048 + 512 * hh + 512],
                        start=True,
                        stop=True,
                    )
                    osl = o_c[:, 512 * hh : 512 * hh + 512]
                    if hh == 0:
                        nc.vector.tensor_scalar_mul(osl, op, r_t)
                    else:
                        nc.scalar.activation(osl, op, AF.Copy, scale=r_t)

                # scatter o_c [(a,i),(c,p)] -> ost [c,(a,i,p)] via DRAM bounce
                ob = dpool.tile([128, 1024], BF16, tag="ob")
                nc.scalar.dma_start(ob, o_c)
                ost = stg.tile([16, 8192], BF16, tag="ost")
                nc.sync.dma_start(
                    ost,
                    ob.rearrange("(a i) (c p) -> c (a i) p", a=8, i=16, c=16, p=64),
                )
                state[("ost", u)] = ost

            def stage_c(u):
                # conv2 (residual + b2 added on host)
                ost = state.pop(("ost", u))
                osum = xio.tile([64, 8192], BF16, tag="os")
                for t in range(8):
                    c2 = pcv.tile([64, 1024], F32, tag="cv")
                    for hh in range(2):
                        nc.tensor.matmul(
                            c2[:, 512 * hh : 512 * hh + 512],
                            w2_t,
                            ost[:, 1024 * t + 512 * hh : 1024 * t + 512 * hh + 512],
                            start=True,
                            stop=True,
                        )
                    osl = osum[:, 1024 * t : 1024 * t + 1024]
                    if t % 2 == 0:
                        nc.vector.tensor_copy(osl, c2)
                    else:
                        nc.scalar.activation(osl, c2, AF.Copy)
                nc.sync.dma_start(out[:, 8192 * u : 8192 * (u + 1)], osum)

            # software pipeline: prefetch | conv1+gather | attention | conv2
            for s in range(units + 3):
                if s < units:
                    stage_in(s)
                if 1 <= s < units + 1:
                    stage_a(s - 1)
                if 2 <= s < units + 2:
                    stage_b(s - 2)
                if 3 <= s:
                    stage_c(s - 3)
    nc.compile()
    return nc


def kernel(x, w_ptg, b_ptg, w_out, b_out):
    x = np.asarray(x, dtype=np.float32)
    w_ptg = np.asarray(w_ptg, dtype=np.float32)
    b_ptg = np.asarray(b_ptg, dtype=np.float32)
    w_out = np.asarray(w_out, dtype=np.float32)
    b_out = np.asarray(b_out, dtype=np.float32)

    # pad vector: w_ptg @ xpad + b_ptg = 0
    xpad, *_ = np.linalg.lstsq(w_ptg, -b_ptg, rcond=None)
    xp = np.empty((4, 64, 512, 512), np.float32)
    xp[:] = xpad.astype(np.float32)[None, :, None, None]
    xp[:, :, :504, :504] = x

    mc = np.zeros((8, 128), dtype=BF)
    for a in range(8):
        mc[a, 16 * a : 16 * a + 16] = 100.0

    common = {
        "w1t": np.ascontiguousarray(w_ptg.T).astype(BF),
        "b1": np.ascontiguousarray(b_ptg[:, None]).astype(np.float32),
        "w2t": np.ascontiguousarray(w_out.T).astype(BF),
        "identb": np.eye(128, dtype=BF),
        "mbu": np.full((1, 128), -100.0, dtype=BF),
        "mbv": np.full((1, 128), 100.0, dtype=BF),
        "mbc": mc,
    }
    in_maps = []
    for b in range(4):
        for h in range(2):
            xb = xp[b, :, 256 * h : 256 * h + 256, :].reshape(
                64, 8, 4, 8, 2, 8, 4, 8
            )
            xb = xb.transpose(0, 1, 4, 5, 2, 6, 3, 7).reshape(64, 131072)
            in_maps.append({"xs": np.ascontiguousarray(xb.astype(BF)), **common})

    if "nc" not in _cached:
        _cached["nc"] = _build_nc()
    res = run_bass_kernel_spmd(_cached["nc"], in_maps, list(range(8)))
    _cached["last_res"] = res

    outp = np.empty((4, 64, 512, 512), np.float32)
    for idx in range(8):
        b, h = divmod(idx, 2)
        o = np.asarray(res.results[idx]["out"]).astype(np.float32)
        o = (
            o.reshape(64, 8, 2, 8, 4, 4, 8, 8)
            .transpose(0, 1, 4, 6, 2, 3, 5, 7)
            .reshape(64, 256, 512)
        )
        outp[b, :, 256 * h : 256 * h + 256, :] = o
    out = outp[:, :, :504, :504] + b_out[None, :, None, None] + x
    return np.ascontiguousarray(out.astype(np.float32))


if __name__ == "__main__":
    import reference

    inputs = {k: np.asarray(v) for k, v in reference.setup_inputs().items()}
    got = kernel(**inputs)
    exp = np.asarray(reference.reference(**inputs))
    err = np.abs(got - exp).max() / np.abs(exp).max()
    print("Relative error:", err)


# revision 23
# speedup vs baseline: 7.0552x; 1.2098x over previous
import numpy as np
import ml_dtypes
import concourse.bass as bass
import concourse.tile as tile
from concourse import bacc, mybir
from concourse.bass_utils import run_bass_kernel_spmd

# Block self-attention: 32x32 areas of 4x4 blocks of 8x8 pixels.
# Sharding: 8 cores = 4 batches x 2 H-halves of 256 rows.
# Host passes x in block-major bf16 layout [64, 16 units x 8192] where a
# unit is 8 areas (one 32-row x 256-col half strip) laid out (area, block,
# pixel). Kernel returns conv2(attention(conv1(x))) in the same layout;
# host adds the residual x and the output bias b2, then un-permutes.
# Padding trick: host pads x spatially with the vector xpad solving
# w_ptg @ xpad + b_ptg = 0, so conv1 output is exactly 0 at padded pixels
# (matches reference, which zero-pads after conv+bias).
# Softmax mask (block-diagonal per area) is applied by accumulating
# -1e4 + 1e4*blockdiag onto the scores via two extra rank-1/rank-8
# matmuls before the exp.

F32 = mybir.dt.float32
BF16 = mybir.dt.bfloat16
AF = mybir.ActivationFunctionType
BF = ml_dtypes.bfloat16

_cached = {}


def _build_nc(units=16):
    nc = bacc.Bacc("TRN2", target_bir_lowering=False, debug=False, num_devices=8)
    xs = nc.dram_tensor("xs", [64, units * 8192], BF16, kind="ExternalInput").ap()
    w1t = nc.dram_tensor("w1t", [128, 48], BF16, kind="ExternalInput").ap()
    b1 = nc.dram_tensor("b1", [48, 1], F32, kind="ExternalInput").ap()
    w2t = nc.dram_tensor("w2t", [48, 64], BF16, kind="ExternalInput").ap()
    identb = nc.dram_tensor("identb", [128, 128], BF16, kind="ExternalInput").ap()
    mbu = nc.dram_tensor("mbu", [9, 128], BF16, kind="ExternalInput").ap()
    mbv = nc.dram_tensor("mbv", [9, 128], BF16, kind="ExternalInput").ap()
    out = nc.dram_tensor("out", [64, units * 8192], BF16, kind="ExternalOutput").ap()

    with tile.TileContext(nc) as tc:
        with (
            tc.tile_pool(name="const", bufs=1) as cpool,
            tc.tile_pool(name="xio", bufs=2) as xio,
            tc.tile_pool(name="stage", bufs=2) as stg,
            tc.tile_pool(name="sm", bufs=2) as sm,
            tc.tile_pool(name="dbounce", bufs=2, space="DRAM") as dpool,
            tc.tile_pool(name="pcv", bufs=2, space="PSUM") as pcv,
            tc.tile_pool(name="ptp", bufs=2, space="PSUM") as ptp,
            tc.tile_pool(name="patt", bufs=2, space="PSUM") as patt,
        ):
            w1_t = cpool.tile([128, 48], BF16)
            nc.sync.dma_start(w1_t, w1t)
            b1_t = cpool.tile([48, 1], F32)
            nc.sync.dma_start(b1_t, b1)
            w2_t = cpool.tile([48, 64], BF16)
            nc.sync.dma_start(w2_t, w2t)
            id_t = cpool.tile([128, 128], BF16)
            nc.sync.dma_start(id_t, identb)
            mbu_t = cpool.tile([9, 128], BF16)
            nc.sync.dma_start(mbu_t, mbu)
            mbv_t = cpool.tile([9, 128], BF16)
            nc.sync.dma_start(mbv_t, mbv)

            state = {}

            def stage_in(u):
                # x2[h*64+c, s*512+w] = xs[c, (2s+h)*512+w]: even/odd 512-col
                # chunks stacked on the partition axis for 2-way row-packed
                # conv1 (PE rows 0-63 and 64-127 run concurrently).
                x_t = xio.tile([128, 4096], BF16, tag="x")
                xsv = xs[:, 8192 * u : 8192 * (u + 1)].rearrange(
                    "c (s h w) -> h c s w", s=8, h=2, w=512
                )
                nc.sync.dma_start(x_t[0:64, :], xsv[0])
                nc.sync.dma_start(x_t[64:128, :], xsv[1])
                state[("x", u)] = x_t

            def stage_a(u):
                # conv1: y = w1 @ x (+b1 in the copy), bf16
                x_t = state.pop(("x", u))
                y_t = stg.tile([48, 8192], BF16, tag="y")
                for t in range(8):
                    c1 = pcv.tile([64, 1024], F32, tag="cv")
                    nc.tensor.matmul(
                        c1[0:48, 0:512],
                        w1_t[0:64, :],
                        x_t[0:64, 512 * t : 512 * t + 512],
                        start=True,
                        stop=True,
                    )
                    nc.tensor.matmul(
                        c1[0:48, 512:1024],
                        w1_t[64:128, :],
                        x_t[64:128, 512 * t : 512 * t + 512],
                        start=True,
                        stop=True,
                    )
                    ysl = y_t[:, 1024 * t : 1024 * t + 1024]
                    if t % 2 == 0:
                        nc.scalar.activation(ysl, c1[0:48, :], AF.Identity, bias=b1_t)
                    else:
                        nc.vector.tensor_scalar_add(ysl, c1[0:48, :], b1_t)

                # gather y [c,(a,i,p)] -> qg [(a,i),(c,p)] via DRAM bounce:
                # contiguous spill (scalar ring), permuted read (sync ring).
                # SBUF-side DMA APs must keep the partition dim outermost;
                # DRAM APs are flat and take any stride pattern.
                yb = dpool.tile([48, 8192], BF16, tag="yb")
                nc.scalar.dma_start(yb, y_t, max_dma_last_dim=2048)
                qg = stg.tile([128, 3072], BF16, tag="qg")
                nc.sync.dma_start(
                    qg,
                    yb.rearrange("c (a i p) -> (a i) c p", a=8, i=16, p=64),
                )
                state[("qg", u)] = qg

            def stage_b(u):
                qg = state.pop(("qg", u))
                # transpose q/t chunks: qkb [(c,p), ai]
                qkb = stg.tile([128, 2048], BF16, tag="qkb")
                for g4 in range(4):
                    tp = ptp.tile([128, 512], BF16, tag="tp")
                    for q in range(4):
                        nc.tensor.transpose(
                            tp[:, 128 * q : 128 * q + 128],
                            qg[:, 512 * g4 + 128 * q : 512 * g4 + 128 * q + 128],
                            id_t,
                        )
                    dst = qkb[:, 512 * g4 : 512 * g4 + 512]
                    if g4 % 2 == 0:
                        nc.vector.tensor_copy(dst, tp)
                    else:
                        nc.scalar.activation(dst, tp, AF.Copy)

                # scores (8 areas batched; mask applied via bias matmuls)
                sps = patt.tile([128, 512], F32, tag="att")
                s_ap = sps[:, 0:128]
                for k in range(8):
                    nc.tensor.matmul(
                        s_ap,
                        qkb[:, 128 * k : 128 * k + 128],
                        qkb[:, 1024 + 128 * k : 1024 + 128 * k + 128],
                        start=(k == 0),
                        stop=False,
                    )
                nc.tensor.matmul(s_ap, mbu_t, mbv_t, start=False, stop=True)

                e_t = sm.tile([128, 128], BF16, tag="e")
                nc.scalar.activation(e_t, s_ap, AF.Exp)
                r_t = sm.tile([128, 1], F32, tag="r")
                nc.vector.reduce_sum(r_t, e_t, axis=mybir.AxisListType.X)
                nc.vector.reciprocal(r_t, r_t)

                etp = ptp.tile([128, 512], BF16, tag="tp")
                nc.tensor.transpose(etp[:, 0:128], e_t, id_t)
                eT = sm.tile([128, 128], BF16, tag="eT")
                nc.vector.tensor_copy(eT, etp[:, 0:128])

                # o = (e @ g) * recip(rowsum) ; normalization in the copy
                o_c = stg.tile([128, 1024], BF16, tag="oc")
                for hh in range(2):
                    op = patt.tile([128, 512], F32, tag="att")
                    nc.tensor.matmul(
                        op,
                        eT,
                        qg[:, 2048 + 512 * hh : 2048 + 512 * hh + 512],
                        start=True,
                        stop=True,
                    )
                    osl = o_c[:, 512 * hh : 512 * hh + 512]
                    if hh == 0:
                        nc.vector.tensor_scalar_mul(osl, op, r_t)
                    else:
                        nc.scalar.activation(osl, op, AF.Copy, scale=r_t)

                # scatter o_c [(a,i),(c,p)] -> ost [c,(a,i,p)] via DRAM bounce
                # ob layout: rows 64h+8s+i hold o[s, h, i, (c, p)]; the
                # h-split lives in the (flat) DRAM-side AP so the SBUF read
                # stays a plain contiguous tile.
                ob = dpool.tile([128, 1024], BF16, tag="ob")
                nc.scalar.dma_start(ob, o_c)
                # ost[c,(a,i,p)] with a duplicate copy at rows 32-47 so conv2
                # can run 2-way row-packed (bases 0 and 32).
                ost = stg.tile([48, 8192], BF16, tag="ost")
                obv = ob.rearrange("(a i) (c p) -> c (a i) p", a=8, i=16, c=16, p=64)
                nc.sync.dma_start(ost[0:16, :], obv)
                nc.sync.dma_start(ost[32:48, :], obv)
                state[("ost", u)] = ost

            def stage_c(u):
                # conv2 (residual + b2 added on host)
                ost = state.pop(("ost", u))
                osum = xio.tile([64, 8192], BF16, tag="os")
                for t in range(8):
                    c2 = pcv.tile([64, 1024], F32, tag="cv")
                    nc.tensor.matmul(
                        c2[:, 0:512],
                        w2_t[0:16, :],
                        ost[0:16, 1024 * t : 1024 * t + 512],
                        start=True,
                        stop=True,
                    )
                    nc.tensor.matmul(
                        c2[:, 512:1024],
                        w2_t[32:48, :],
                        ost[32:48, 1024 * t + 512 : 1024 * t + 1024],
                        start=True,
                        stop=True,
                    )
                    osl = osum[:, 1024 * t : 1024 * t + 1024]
                    if t % 2 == 0:
                        nc.vector.tensor_copy(osl, c2)
                    else:
                        nc.scalar.activation(osl, c2, AF.Copy)
                nc.sync.dma_start(
                    out[:, 8192 * u : 8192 * (u + 1)], osum, max_dma_last_dim=2048
                )

            # software pipeline: prefetch | conv1+gather | attention | conv2
            for s in range(units + 3):
                if s < units:
                    stage_in(s)
                if 1 <= s < units + 1:
                    stage_a(s - 1)
                if 2 <= s < units + 2:
                    stage_b(s - 2)
                if 3 <= s:
                    stage_c(s - 3)
    nc.compile()
    return nc


def kernel(x, w_ptg, b_ptg, w_out, b_out):
    x = np.asarray(x, dtype=np.float32)
    w_ptg = np.asarray(w_ptg, dtype=np.float32)
    b_ptg = np.asarray(b_ptg, dtype=np.float32)
    w_out = np.asarray(w_out, dtype=np.float32)
    b_out = np.asarray(b_out, dtype=np.float32)

    # pad vector: w_ptg @ xpad + b_ptg = 0
    xpad, *_ = np.linalg.lstsq(w_ptg, -b_ptg, rcond=None)
    xp = np.empty((4, 64, 512, 512), np.float32)
    xp[:] = xpad.astype(np.float32)[None, :, None, None]
    xp[:, :, :504, :504] = x

    mc = np.zeros((8, 128), dtype=BF)
    for a in range(8):
        mc[a, 16 * a : 16 * a + 16] = 100.0
    mbu = np.concatenate([np.full((1, 128), -100.0, dtype=BF), mc])
    mbv = np.concatenate([np.full((1, 128), 100.0, dtype=BF), mc])

    w1d = np.concatenate([w_ptg.T, w_ptg.T]).astype(BF)  # [128, 48]
    w2d = np.zeros((48, 64), dtype=BF)
    w2d[0:16] = w_out.T.astype(BF)
    w2d[32:48] = w_out.T.astype(BF)

    common = {
        "w1t": np.ascontiguousarray(w1d),
        "b1": np.ascontiguousarray(b_ptg[:, None]).astype(np.float32),
        "w2t": w2d,
        "identb": np.eye(128, dtype=BF),
        "mbu": mbu,
        "mbv": mbv,
    }
    in_maps = []
    for b in range(4):
        for h in range(2):
            xb = xp[b, :, 256 * h : 256 * h + 256, :].reshape(
                64, 8, 4, 8, 2, 8, 4, 8
            )
            xb = xb.transpose(0, 1, 4, 5, 2, 6, 3, 7).reshape(64, 131072)
            in_maps.append({"xs": np.ascontiguousarray(xb.astype(BF)), **common})

    if "nc" not in _cached:
        _cached["nc"] = _build_nc()
    res = run_bass_kernel_spmd(_cached["nc"], in_maps, list(range(8)))
    _cached["last_res"] = res

    outp = np.empty((4, 64, 512, 512), np.float32)
    for idx in range(8):
        b, h = divmod(idx, 2)
        o = np.asarray(res.results[idx]["out"]).astype(np.float32)
        o = (
            o.reshape(64, 8, 2, 8, 4, 4, 8, 8)
            .transpose(0, 1, 4, 6, 2, 3, 5, 7)
            .reshape(64, 256, 512)
        )
        outp[b, :, 256 * h : 256 * h + 256, :] = o
    out = outp[:, :, :504, :504] + b_out[None, :, None, None] + x
    return np.ascontiguousarray(out.astype(np.float32))


if __name__ == "__main__":
    import reference

    inputs = {k: np.asarray(v) for k, v in reference.setup_inputs().items()}
    got = kernel(**inputs)
    exp = np.asarray(reference.reference(**inputs))
    err = np.abs(got - exp).max() / np.abs(exp).max()
    print("Relative error:", err)
